# revision 22
# baseline (speedup 1.0000x reference)
"""Trainium2 Bass kernel for the attention LSTM decoder (nn_Decoder).

Strategy: data-parallel over batch N=128 across 8 cores (16 batch each).
Each core runs an independent 256-step teacher-forced decode chain:
  - LSTM matmuls: weights stationary (bf16, FWL), gates on PSUM partitions,
    batch (16) on the moving free dim.
  - attention: per-(n, tchunk) key/value-stationary matmuls; softmax sum via
    PE ones-reduction over the partition (t) dim; no max-subtraction
    (masked entries get -30000 added before exp).
  - sigmoid via tanh: sigma(x) = 0.5*tanh(x/2)+0.5, with g-gate weight rows
    pre-scaled x2 so one tanh(scale=0.5) ACT covers all 4 gates. All
    transcendentals (tanh, exp) live in one ACT table set -> no set switches.
  - c-state stored as S = 2c so all 0.5 factors fold into ACT scale / DVE ops.
"""

import os

import numpy as np
import ml_dtypes

import concourse.bacc as bacc
import concourse.bass as bass
import concourse.mybir as mybir
import concourse.tile as tile
from concourse.bass_utils import run_bass_kernel_spmd

T_ENC = 512
NB = 128
L_FULL = 256
KV = 128
VS = 128
EMB = 256
HID = 512
VOC = 34
NCORES = 8
NL = NB // NCORES          # batch per core = 16
TC = T_ENC // 128          # 4 t-chunks
P = 128

F32 = mybir.dt.float32
BF16 = mybir.dt.bfloat16
AF = mybir.ActivationFunctionType
OP = mybir.AluOpType
BF16_NP = ml_dtypes.bfloat16

MASK_NEG = -30000.0


def _gate_perm(h):
    # pytorch gate rows [i, f, g, o] -> [i, f, o, g]
    return np.concatenate([
        np.arange(0, h), np.arange(h, 2 * h),
        np.arange(3 * h, 4 * h), np.arange(2 * h, 3 * h),
    ])


def _kernel_body(nc, tc_ctx, io, L, reps=1, cnt=(TC,) * NL):
    tcx = tc_ctx
    import contextlib

    with contextlib.ExitStack() as stack:
        const = stack.enter_context(tcx.tile_pool(name="const", bufs=1))
        work = stack.enter_context(tcx.tile_pool(name="work", bufs=2))
        pg1 = stack.enter_context(tcx.tile_pool(name="pg1", bufs=1, space="PSUM"))
        pg2 = stack.enter_context(tcx.tile_pool(name="pg2", bufs=1, space="PSUM"))
        pE = stack.enter_context(tcx.tile_pool(name="pE", bufs=1, space="PSUM"))
        psm = stack.enter_context(tcx.tile_pool(name="psm", bufs=1, space="PSUM"))
        pcx = stack.enter_context(tcx.tile_pool(name="pcx", bufs=1, space="PSUM"))
        ppd = stack.enter_context(tcx.tile_pool(name="ppd", bufs=1, space="PSUM"))

        # ---- constant SBUF tensors (loaded once) ----
        key_stat = const.tile([P, NL, TC, P], BF16, tag="key_stat")
        val_stat = const.tile([P, NL, TC, P], BF16, tag="val_stat")
        w1_stat = const.tile([P, 5, 16, P], BF16, tag="w1_stat")
        w2_stat = const.tile([P, 5, 4, P], BF16, tag="w2_stat")
        wout_stat = const.tile([P, 2, VOC], BF16, tag="wout_stat")
        b2_row = const.tile([1, 4, P], BF16, tag="b2_row")
        bout_row = const.tile([1, VOC], BF16, tag="bout_row")
        onehot = const.tile([VOC + 1, L, NL], BF16, tag="onehot")
        maskneg = const.tile([P, NL, TC], F32, tag="maskneg")
        pre_stat = const.tile([VOC + 1, 16, P], BF16, tag="pre_stat")
        staging = const.tile([VOC, L, NL], F32, tag="staging")

        ones16 = const.tile([1, NL], BF16, tag="ones16")
        ones_col = const.tile([P, 1], BF16, tag="ones_col")

        # persistent states
        s1 = const.tile([P, 4, NL], F32, tag="s1")       # 2*c1
        s2 = const.tile([P, 1, NL], F32, tag="s2")       # 2*c2
        h1T = const.tile([P, 4, NL], BF16, tag="h1T")
        h2T = const.tile([P, NL], BF16, tag="h2T")
        ctxT = const.tile([P, NL], BF16, tag="ctxT")

        # ---- prologue: DMA inputs ----
        nc.sync.dma_start(out=key_stat, in_=io["key_stat"].ap())
        nc.sync.dma_start(out=val_stat, in_=io["val_stat"].ap())
        nc.sync.dma_start(out=w1_stat, in_=io["w1_stat"].ap())
        nc.sync.dma_start(out=w2_stat, in_=io["w2_stat"].ap())
        nc.sync.dma_start(out=wout_stat, in_=io["wout_stat"].ap())
        nc.sync.dma_start(out=b2_row, in_=io["b2_row"].ap())
        nc.sync.dma_start(out=bout_row, in_=io["bout_row"].ap())
        nc.sync.dma_start(out=onehot, in_=io["onehot"].ap())
        nc.sync.dma_start(out=maskneg, in_=io["maskneg"].ap())

        nc.vector.memset(ones16, 1.0)
        nc.vector.memset(ones_col, 1.0)
        nc.vector.memset(s1, 0.0)
        nc.vector.memset(s2, 0.0)
        nc.vector.memset(h1T, 0.0)
        nc.vector.memset(h2T, 0.0)
        nc.vector.memset(ctxT, 0.0)

        # ---- prologue: PRE = emb_ext.T @ w1e_rhs  -> (35, 2048) bf16 ----
        with tcx.tile_pool(name="prep", bufs=1) as prep, \
             tcx.tile_pool(name="prepp", bufs=1, space="PSUM") as prepp:
            emb_a = prep.tile([P, VOC + 1], F32, tag="emb_a")
            emb_b = prep.tile([P, VOC + 1], F32, tag="emb_b")
            emb_c = prep.tile([1, VOC + 1], F32, tag="emb_c")
            rhs_a = prep.tile([P, 2048], F32, tag="rhs_a")
            rhs_b = prep.tile([P, 2048], F32, tag="rhs_b")
            rhs_c = prep.tile([1, 2048], F32, tag="rhs_c")
            nc.sync.dma_start(out=emb_a, in_=io["emb_ext"].ap()[0:P, :])
            nc.sync.dma_start(out=emb_b, in_=io["emb_ext"].ap()[P:2 * P, :])
            nc.sync.dma_start(out=emb_c, in_=io["emb_ext"].ap()[2 * P:2 * P + 1, :])
            nc.sync.dma_start(out=rhs_a, in_=io["w1e_rhs"].ap()[0:P, :])
            nc.sync.dma_start(out=rhs_b, in_=io["w1e_rhs"].ap()[P:2 * P, :])
            nc.sync.dma_start(out=rhs_c, in_=io["w1e_rhs"].ap()[2 * P:2 * P + 1, :])
            for nf in range(4):
                pp = prepp.tile([VOC + 1, 512], F32, tag="prepsum")
                sl = slice(nf * 512, (nf + 1) * 512)
                nc.tensor.matmul(pp, emb_a, rhs_a[:, sl], start=True, stop=False)
                nc.tensor.matmul(pp, emb_b, rhs_b[:, sl], start=False, stop=False)
                nc.tensor.matmul(pp, emb_c, rhs_c[:, sl], start=False, stop=True)
                # cast to bf16 stationary: pre_stat free = (gtile, m)
                nc.scalar.copy(
                    out=pre_stat[:, nf * 4:(nf + 1) * 4, :], in_=pp)

        # attention (slot, tchunk) work list: slots with short sequences skip
        # fully-masked chunks (their expe cols are exactly 0 via maskneg)
        att_list = [(s, c) for s in range(NL) for c in range(cnt[s])]
        ep = pE.tile([P, NL, TC], F32, tag="ep")     # persistent; skipped
        nc.vector.memset(ep, 0.0)                    # cols stay 0 forever

        # ---- decode loop ----
        for t in range(L * reps):
            t = t % L
            # ============ LSTM cell 1 ============
            g1p = pg1.tile([P, 16, NL], F32, tag="g1p")
            oh_t = onehot[:, t, :]
            for g in range(16):
                nc.tensor.matmul(g1p[:, g, :], pre_stat[:, g, :], oh_t,
                                 start=(g == 0), stop=False)
            for c in range(4):
                for g in range(16):
                    nc.tensor.matmul(g1p[:, g, :], w1_stat[:, 1 + c, g, :],
                                     h1T[:, c, :], start=False, stop=False)
            for g in range(16):
                nc.tensor.matmul(g1p[:, g, :], w1_stat[:, 0, g, :], ctxT,
                                 start=False, stop=(g == 15))

            t1 = work.tile([P, 16, NL], F32, tag="t1")
            nc.scalar.activation(t1, g1p, AF.Tanh, scale=0.5)

            sgf = work.tile([P, 4, NL], F32, tag="sgf")
            tA = work.tile([P, 4, NL], F32, tag="tA")
            tB = work.tile([P, 4, NL], F32, tag="tB")
            tD = work.tile([P, 4, NL], F32, tag="tD")
            th1 = work.tile([P, 4, NL], F32, tag="th1")
            sgo = work.tile([P, 4, NL], F32, tag="sgo")
            nc.vector.tensor_scalar(sgf, t1[:, 4:8, :], 0.5, 0.5,
                                    op0=OP.mult, op1=OP.add)
            nc.vector.tensor_mul(tA, sgf, s1)
            nc.vector.tensor_mul(tB, t1[:, 0:4, :], t1[:, 12:16, :])
            nc.vector.tensor_add(tD, tB, t1[:, 12:16, :])
            nc.vector.tensor_add(s1, tA, tD)
            nc.scalar.activation(th1, s1, AF.Tanh, scale=0.5)
            nc.vector.tensor_scalar(sgo, t1[:, 8:12, :], 0.5, 0.5,
                                    op0=OP.mult, op1=OP.add)
            nc.vector.tensor_mul(h1T, sgo, th1)

            # ============ LSTM cell 2 ============
            g2p = pg2.tile([P, 4, NL], F32, tag="g2p")
            for g in range(4):
                nc.tensor.matmul(g2p[:, g, :], b2_row[:, g, :], ones16,
                                 start=(g == 0), stop=False)
            for g in range(4):
                nc.tensor.matmul(g2p[:, g, :], w2_stat[:, 0, g, :], h2T,
                                 start=False, stop=False)
            for c in range(4):
                for g in range(4):
                    nc.tensor.matmul(g2p[:, g, :], w2_stat[:, 1 + c, g, :],
                                     h1T[:, c, :],
                                     start=False, stop=(c == 3 and g == 3))

            t2 = work.tile([P, 4, NL], F32, tag="t2")
            nc.scalar.activation(t2, g2p, AF.Tanh, scale=0.5)

            sgf2 = work.tile([P, 1, NL], F32, tag="sgf2")
            tA2 = work.tile([P, 1, NL], F32, tag="tA2")
            tB2 = work.tile([P, 1, NL], F32, tag="tB2")
            tD2 = work.tile([P, 1, NL], F32, tag="tD2")
            th2 = work.tile([P, 1, NL], F32, tag="th2")
            sgo2 = work.tile([P, 1, NL], F32, tag="sgo2")
            nc.vector.tensor_scalar(sgf2, t2[:, 1:2, :], 0.5, 0.5,
                                    op0=OP.mult, op1=OP.add)
            nc.vector.tensor_mul(tA2, sgf2, s2)
            nc.vector.tensor_mul(tB2, t2[:, 0:1, :], t2[:, 3:4, :])
            nc.vector.tensor_add(tD2, tB2, t2[:, 3:4, :])
            nc.vector.tensor_add(s2, tA2, tD2)
            nc.scalar.activation(th2, s2, AF.Tanh, scale=0.5)
            nc.vector.tensor_scalar(sgo2, t2[:, 2:3, :], 0.5, 0.5,
                                    op0=OP.mult, op1=OP.add)
            nc.vector.tensor_mul(h2T, sgo2[:, 0, :], th2[:, 0, :])

            # ============ attention ============
            for i, (n, tcc) in enumerate(att_list):
                nc.tensor.matmul(ep[:, n, tcc:tcc + 1],
                                 key_stat[:, n, tcc, :],
                                 h2T[:, n:n + 1],
                                 start=(i == 0),
                                 stop=(i == len(att_list) - 1))
            esb = work.tile([P, NL, TC], F32, tag="esb")
            nc.vector.tensor_add(esb, ep, maskneg)
            expe = work.tile([P, NL, TC], BF16, tag="expe")
            nc.scalar.activation(expe, esb, AF.Exp)

            sums_p = psm.tile([1, NL, TC], F32, tag="sums_p")
            nc.tensor.matmul(sums_p, ones_col, expe, start=True, stop=True)
            sums_n = work.tile([1, NL], F32, tag="sums_n")
            nc.vector.reduce_sum(out=sums_n, in_=sums_p,
                                 axis=mybir.AxisListType.X)
            sums_b = work.tile([P, NL], F32, tag="sums_b")
            nc.gpsimd.partition_broadcast(sums_b, sums_n)
            recip_b = work.tile([P, NL], F32, tag="recip_b")
            nc.vector.reciprocal(out=recip_b, in_=sums_b)

            cxp = pcx.tile([P, NL], F32, tag="cxp")
            for i, (n, tcc) in enumerate(att_list):
                nc.tensor.matmul(cxp[:, n:n + 1],
                                 val_stat[:, n, tcc, :],
                                 expe[:, n, tcc:tcc + 1],
                                 start=(i == 0),
                                 stop=(i == len(att_list) - 1))
            nc.vector.tensor_mul(ctxT, cxp, recip_b)

            # ============ output head ============
            pp = ppd.tile([VOC, NL], F32, tag="pp")
            nc.tensor.matmul(pp, bout_row, ones16, start=True, stop=False)
            nc.tensor.matmul(pp, wout_stat[:, 0, :], h2T, start=False,
                             stop=False)
            nc.tensor.matmul(pp, wout_stat[:, 1, :], ctxT, start=False,
                             stop=True)
            nc.vector.tensor_copy(staging[:, t, :], pp)

        # ---- epilogue ----
        nc.sync.dma_start(out=io["out"].ap(), in_=staging)


def build_module(L=L_FULL, reps=1, cnt=(TC,) * NL):
    nc = bacc.Bacc("TRN2", target_bir_lowering=False, debug=False)
    io = {}
    io["key_stat"] = nc.dram_tensor("key_stat", [P, NL, TC, P], BF16,
                                    kind="ExternalInput")
    io["val_stat"] = nc.dram_tensor("val_stat", [P, NL, TC, P], BF16,
                                    kind="ExternalInput")
    io["w1_stat"] = nc.dram_tensor("w1_stat", [P, 5, 16, P], BF16,
                                   kind="ExternalInput")
    io["w2_stat"] = nc.dram_tensor("w2_stat", [P, 5, 4, P], BF16,
                                   kind="ExternalInput")
    io["wout_stat"] = nc.dram_tensor("wout_stat", [P, 2, VOC], BF16,
                                     kind="ExternalInput")
    io["b2_row"] = nc.dram_tensor("b2_row", [1, 4, P], BF16,
                                  kind="ExternalInput")
    io["bout_row"] = nc.dram_tensor("bout_row", [1, VOC], BF16,
                                    kind="ExternalInput")
    io["onehot"] = nc.dram_tensor("onehot", [VOC + 1, L, NL], BF16,
                                  kind="ExternalInput")
    io["maskneg"] = nc.dram_tensor("maskneg", [P, NL, TC], F32,
                                   kind="ExternalInput")
    io["emb_ext"] = nc.dram_tensor("emb_ext", [EMB + 1, VOC + 1], F32,
                                   kind="ExternalInput")
    io["w1e_rhs"] = nc.dram_tensor("w1e_rhs", [EMB + 1, 4 * HID], F32,
                                   kind="ExternalInput")
    io["out"] = nc.dram_tensor("out", [VOC, L, NL], F32,
                               kind="ExternalOutput")

    with tile.TileContext(nc) as tcx:
        _kernel_body(nc, tcx, io, L, reps=reps, cnt=cnt)
    nc.compile()
    return nc


def batch_assignment(lens):
    """Deal batch elements (sorted by length desc) round-robin to cores.

    orig index ranks[s*NCORES + c] -> core c, slot s. All cores share slot s's
    chunk count cnt[s] (the max in the slot group), so one SPMD program fits
    every core and per-core work is balanced.
    """
    lens = np.asarray(lens).astype(np.int64)
    ranks = np.argsort(-lens, kind="stable")
    cnt = tuple(int(-(-lens[ranks[s * NCORES]] // 128)) for s in range(NL))
    return ranks, cnt


def make_in_maps(key, values, lens, text, emb, W_ih1, W_hh1, b_ih1, b_hh1,
                 W_ih2, W_hh2, b_ih2, b_hh2, W_out, b_out, L=L_FULL):
    key = np.asarray(key, np.float32)
    values = np.asarray(values, np.float32)
    lens = np.asarray(lens).astype(np.int64)
    text = np.asarray(text).astype(np.int64)
    emb = np.asarray(emb, np.float32)

    perm1 = _gate_perm(HID)
    W1 = np.concatenate([np.asarray(W_ih1, np.float32),
                         np.asarray(W_hh1, np.float32)], axis=1)[perm1]
    b1 = (np.asarray(b_ih1, np.float32) + np.asarray(b_hh1, np.float32))[perm1]
    W1 = W1.copy()
    b1 = b1.copy()
    W1[3 * HID:] *= 2.0      # g-gate rows x2 (tanh trick)
    b1[3 * HID:] *= 2.0

    perm2 = _gate_perm(KV)
    W2 = np.concatenate([np.asarray(W_ih2, np.float32),
                         np.asarray(W_hh2, np.float32)], axis=1)[perm2]
    b2 = (np.asarray(b_ih2, np.float32) + np.asarray(b_hh2, np.float32))[perm2]
    W2 = W2.copy()
    b2 = b2.copy()
    W2[3 * KV:] *= 2.0
    b2[3 * KV:] *= 2.0

    Wo = np.asarray(W_out, np.float32)
    bo = np.asarray(b_out, np.float32)

    # shared (same on all cores)
    emb_ext = np.zeros((EMB + 1, VOC + 1), np.float32)
    emb_ext[:EMB, :VOC] = emb.T
    emb_ext[EMB, VOC] = 1.0
    w1e_rhs = np.concatenate([W1[:, :EMB].T, b1[None, :]], axis=0)  # (257,2048)

    # w1_stat[k, c, g, m]: c=0 ctx (W1 cols 256:384), c=1..4 h1 (384:896)
    w1ct = W1[:, EMB:EMB + VS].T.reshape(P, 16, P)             # [k, g, m]
    w1h = W1[:, EMB + VS:].T.reshape(4, P, 16, P)              # [c, k, g, m]
    w1_stat = np.concatenate([w1ct[:, None], w1h.transpose(1, 0, 2, 3)],
                             axis=1).astype(BF16_NP)           # (128,5,16,128)

    w2h2 = W2[:, HID:].T.reshape(P, 4, P)                      # [k, g, m]
    w2h1 = W2[:, :HID].T.reshape(4, P, 4, P)                   # [c, k, g, m]
    w2_stat = np.concatenate([w2h2[:, None], w2h1.transpose(1, 0, 2, 3)],
                             axis=1).astype(BF16_NP)           # (128,5,4,128)
    b2_row = b2.reshape(1, 4, P).astype(BF16_NP)

    wout_stat = np.stack([Wo[:, :KV].T, Wo[:, KV:].T], axis=1).astype(BF16_NP)
    bout_row = bo.reshape(1, VOC).astype(BF16_NP)

    shared = dict(emb_ext=emb_ext, w1e_rhs=w1e_rhs, w1_stat=w1_stat,
                  w2_stat=w2_stat, b2_row=b2_row, wout_stat=wout_stat,
                  bout_row=bout_row)

    ranks, _ = batch_assignment(lens)
    in_maps = []
    tt = np.arange(T_ENC)
    for c in range(NCORES):
        B = ranks[c::NCORES] if ranks is not None else np.arange(
            c * NL, (c + 1) * NL)
        # key_stat[k, n, tc, t] = key[tc*128+t, B[n], k]
        ks = key[:, B, :].reshape(TC, P, NL, KV).transpose(3, 2, 0, 1)
        vs = values[:, B, :].reshape(TC, P, NL, VS).transpose(1, 2, 0, 3)
        # onehot[v, t, n]
        oh = np.zeros((VOC + 1, L, NL), np.float32)
        txt = text[B, :L]                                      # (16, L)
        nn_idx, tt_idx = np.meshgrid(np.arange(NL), np.arange(L),
                                     indexing="ij")
        oh[txt.reshape(-1), tt_idx.reshape(-1), nn_idx.reshape(-1)] = 1.0
        oh[VOC, :, :] = 1.0
        # maskneg[t, n, tc]
        valid = (tt[None, :] < lens[B][:, None])               # (16, 512)
        mk = np.where(valid, 0.0, MASK_NEG).astype(np.float32)
        mk = mk.reshape(NL, TC, P).transpose(2, 0, 1).copy()   # (128, 16, 4)
        in_maps.append(dict(shared,
                            key_stat=ks.astype(BF16_NP).copy(),
                            val_stat=vs.astype(BF16_NP).copy(),
                            onehot=oh.astype(BF16_NP),
                            maskneg=mk))
    return in_maps


_MODULE_CACHE = {}


def kernel(**inputs):
    L = int(np.asarray(inputs["text"]).shape[1])
    ranks, cnt = batch_assignment(inputs["lens"])
    key_c = (L, cnt)
    if key_c not in _MODULE_CACHE:
        _MODULE_CACHE[key_c] = build_module(L, cnt=cnt)
    nc = _MODULE_CACHE[key_c]
    in_maps = make_in_maps(**inputs, L=L)
    res = run_bass_kernel_spmd(nc, in_maps, core_ids=list(range(NCORES)))
    out = np.zeros((NB, L, VOC), np.float32)
    for c in range(NCORES):
        o = res.results[c]["out"]                              # (34, L, 16)
        for s in range(NL):
            out[ranks[s * NCORES + c]] = o[:, :, s].T
    return out
